# revision 9
# baseline (speedup 1.0000x reference)
"""DETR-style matching loss on 8 Trainium2 NeuronCores.

Device (data-parallel over batch, 8 samples/core): the [B,P,T] pairwise
cost matrix  cost = NLL + mask*(5*L1 + 2*GIoU_loss)  computed per core as
a [1200, 150] slab (rows = (b,p) packed, 10 tiles of 128 partitions).
Host: scipy Hungarian per sample on the device cost + final gather/mean
(inherently sequential, O(B*T^3) scalar work on 150x150 matrices).

Math notes (per pair p,t):
  NLL + 4*mask comes from one PE matmul: lhsT rows 0..80 = ln(p).T,
  row 81..127 = 1.0;  rhs rows 0..80 = -onehot(cls), row 81 = 4*mask,
  rows 82..127 = 0.
  iw = min(px1,lx1) - max(px0,lx0)   (unclipped)
  ew = (pwE + lwE) - iw              (exact identity: min+max = sum)
  ue = (area_l + area_p + EPS) - inter = union + EPS
  2*giou_loss = 4 - 2*inter/ue - 2*ue/ee   where ee = enclose + EPS
  cost = [nll + 4*mask] + mask*(1.25*sum|d| - (2*inter/ue + 2*ue/ee))
"""

import sys

import numpy as np

for _p in ("/opt/trn_rl_repo",):
    if _p not in sys.path:
        sys.path.append(_p)

import concourse.bacc as bacc
import concourse.tile as tile
from concourse import mybir
from concourse.bass_utils import run_bass_kernel_spmd

F32 = mybir.dt.float32
EPS = np.float32(1e-7)
IMG_SIZE = np.float32(320.0)
NO_OBJ = 80

B, P, T, C = 64, 150, 150, 81
NCORES = 8
BL = B // NCORES          # 8 samples per core
ROWS = BL * P             # 1200 (b,p) rows per core
NTILES = (ROWS + 127) // 128   # 10

# quantity index -> (psum chunk j, offset within chunk)
# chunks are 512 f32 = one PSUM bank, 3 quantities per chunk
Q_NAMES = ["lx0", "ly0", "lx1", "ly1", "lwE", "lhE",
           "lcx", "lcy", "lwL", "lhL", "areal", "mask"]
NQ = len(Q_NAMES)
NCHUNK = 4


def _tile_segments():
    """Static tile table: for each 128-row tile, the (b, p0, len, part_off)
    segments (at most 2, since P=150 > 128)."""
    tiles = []
    for k in range(NTILES):
        r0 = k * 128
        r1 = min(r0 + 128, ROWS)
        segs = []
        r = r0
        while r < r1:
            b = r // P
            p0 = r % P
            ln = min(r1 - r, P - p0)
            segs.append((b, p0, ln, r - r0))
            r += ln
        tiles.append(segs)
    return tiles


TILES = _tile_segments()

_CACHE = {}

# set by callers that want profiling info
TRACE = False
LAST_EXEC_NS = None
LAST_RESULTS = None


def _build_program():
    nc = bacc.Bacc(None, target_bir_lowering=False)

    pt_d = nc.declare_dram_parameter("pt", [BL, C, P], F32, isOutput=False)
    rhs_d = nc.declare_dram_parameter("rhsnm", [BL, 128, T], F32, isOutput=False)
    qpair_d = nc.declare_dram_parameter("qpair", [NTILES, 2, NCHUNK * 512], F32,
                                        isOutput=False)
    sel_d = nc.declare_dram_parameter("sel", [2, NTILES * 128], F32,
                                      isOutput=False)
    scal_d = nc.declare_dram_parameter("scal", [ROWS, 12], F32, isOutput=False)
    cost_d = nc.declare_dram_parameter("cost", [ROWS, T], F32, isOutput=True)
    nll_d = nc.declare_dram_parameter("nllm", [ROWS, T], F32, isOutput=True)

    from contextlib import ExitStack

    with tile.TileContext(nc) as tc, ExitStack() as ctx:
        consts = ctx.enter_context(tc.tile_pool(name="consts", bufs=1))
        perb = ctx.enter_context(tc.tile_pool(name="perb", bufs=3))
        ptp = ctx.enter_context(tc.tile_pool(name="ptp", bufs=2))
        pert = ctx.enter_context(tc.tile_pool(name="pert", bufs=3))
        work = ctx.enter_context(tc.tile_pool(name="work", bufs=2))
        outp = ctx.enter_context(tc.tile_pool(name="outp", bufs=3))
        bcp = ctx.enter_context(tc.tile_pool(name="bcp", bufs=1, space="PSUM"))
        nllp = ctx.enter_context(tc.tile_pool(name="nllp", bufs=2, space="PSUM"))
        if True:
            sel_t = consts.tile([2, NTILES * 128], F32)
            nc.sync.dma_start(out=sel_t[:], in_=sel_d[:])
            eps_t = consts.tile([128, 1], F32)
            nc.vector.memset(eps_t[:], float(EPS))

            lhs = {}      # b -> [128, P] sbuf tile (ln(p).T over rows 0..80)
            rhs = {}      # b -> [128, T] sbuf tile

            def ensure_b(b):
                if b in lhs:
                    return
                ptile = ptp.tile([C, P], F32, tag="ptile")
                nc.sync.dma_start(out=ptile[:], in_=pt_d[b])
                lt = perb.tile([128, P], F32, tag="lhs")
                nc.vector.memset(lt[64:128, :], 1.0)
                nc.scalar.activation(out=lt[0:C, :], in_=ptile[:],
                                     func=mybir.ActivationFunctionType.Ln)
                rt = perb.tile([128, T], F32, tag="rhs")
                nc.sync.dma_start(out=rt[:], in_=rhs_d[b])
                lhs[b] = lt
                rhs[b] = rt

            for k, segs in enumerate(TILES):
                nrow = sum(s[2] for s in segs)
                for b, _, _, _ in segs:
                    ensure_b(b)

                # label-quantity broadcast tiles (4 psum banks)
                qp = pert.tile([2, NCHUNK * 512], F32, tag="qp")
                nc.sync.dma_start(out=qp[:], in_=qpair_d[k])
                bct = [bcp.tile([128, 512], F32, tag=f"bc{j}", name=f"bc{j}")
                       for j in range(NCHUNK)]
                for j in range(NCHUNK):
                    nc.tensor.matmul(
                        bct[j][:, :],
                        sel_t[:, k * 128:(k + 1) * 128],
                        qp[:, j * 512:(j + 1) * 512],
                        start=True, stop=True,
                    )

                def bc(q):
                    j, off = divmod(q, 3)
                    return bct[j][0:nrow, off * 150:off * 150 + T]

                # nll + 4*mask: one psum tile per segment (matmul psum
                # writes must start at partition 0/32/64)
                nms = []
                for si, (b, p0, ln, off) in enumerate(segs):
                    nm = nllp.tile([128, 512], F32, tag=f"nll{si}",
                                   name=f"nll{si}")
                    nc.tensor.matmul(
                        nm[0:ln, 0:T],
                        lhs[b][:, p0:p0 + ln],
                        rhs[b][:, :],
                        start=True, stop=True,
                    )
                    nsb = outp.tile([128, T], F32, tag=f"nsb{si}",
                                    name=f"nsb{si}")
                    nc.scalar.copy(out=nsb[0:ln, :], in_=nm[0:ln, 0:T])
                    nc.sync.dma_start(
                        out=nll_d[k * 128 + off:k * 128 + off + ln, :],
                        in_=nsb[0:ln, :])
                    nms.append(nm)

                # per-partition pred scalars
                sc = pert.tile([128, 12], F32, tag="scal")
                nc.sync.dma_start(out=sc[0:nrow, :],
                                  in_=scal_d[k * 128:k * 128 + nrow, :])

                def col(i):
                    return sc[0:nrow, i:i + 1]

                (px0, py0, px1, py1, pwE, phE, apE,
                 ncx, ncy, nwL, nhL) = [col(i) for i in range(11)]

                def wt(tag):
                    return work.tile([128, T], F32, tag=tag, name=tag)[0:nrow, :]

                v = nc.vector
                s = nc.scalar
                Ax = wt("Ax"); v.tensor_scalar_min(Ax, bc(2), px1)
                Bx = wt("Bx"); v.tensor_scalar_max(Bx, bc(0), px0)
                Ay = wt("Ay"); v.tensor_scalar_min(Ay, bc(3), py1)
                By = wt("By"); v.tensor_scalar_max(By, bc(1), py0)
                iw = wt("iw"); v.tensor_sub(iw, Ax, Bx)
                ih = wt("ih"); v.tensor_sub(ih, Ay, By)
                iwr = wt("iwr")
                s.activation(out=iwr, in_=iw,
                             func=mybir.ActivationFunctionType.Relu)
                ihr = wt("ihr")
                s.activation(out=ihr, in_=ih,
                             func=mybir.ActivationFunctionType.Relu)
                inter = wt("inter"); v.tensor_mul(inter, iwr, ihr)
                ue = wt("ue")
                v.scalar_tensor_tensor(ue, bc(10), apE, inter,
                                       mybir.AluOpType.add,
                                       mybir.AluOpType.subtract)
                ew = wt("ew")
                v.scalar_tensor_tensor(ew, bc(4), pwE, iw,
                                       mybir.AluOpType.add,
                                       mybir.AluOpType.subtract)
                eh = wt("eh")
                v.scalar_tensor_tensor(eh, bc(5), phE, ih,
                                       mybir.AluOpType.add,
                                       mybir.AluOpType.subtract)
                enc = wt("enc"); v.tensor_mul(enc, ew, eh)
                ee = wt("ee")
                s.activation(out=ee, in_=enc,
                             func=mybir.ActivationFunctionType.Identity,
                             bias=eps_t[0:nrow])
                rue = wt("rue"); v.reciprocal(rue, ue)
                ree = wt("ree"); v.reciprocal(ree, ee)
                iou2 = wt("iou2")
                v.scalar_tensor_tensor(iou2, inter, 2.0, rue,
                                       mybir.AluOpType.mult,
                                       mybir.AluOpType.mult)
                ur2 = wt("ur2")
                v.scalar_tensor_tensor(ur2, ue, 2.0, ree,
                                       mybir.AluOpType.mult,
                                       mybir.AluOpType.mult)
                g = wt("g"); v.tensor_add(g, iou2, ur2)
                d0 = wt("d0")
                s.activation(out=d0, in_=bc(6),
                             func=mybir.ActivationFunctionType.Abs, bias=ncx)
                d1 = wt("d1")
                s.activation(out=d1, in_=bc(7),
                             func=mybir.ActivationFunctionType.Abs, bias=ncy)
                d2 = wt("d2")
                s.activation(out=d2, in_=bc(8),
                             func=mybir.ActivationFunctionType.Abs, bias=nwL)
                d3 = wt("d3")
                s.activation(out=d3, in_=bc(9),
                             func=mybir.ActivationFunctionType.Abs, bias=nhL)
                s01 = wt("s01"); v.tensor_add(s01, d0, d1)
                s23 = wt("s23"); v.tensor_add(s23, d2, d3)
                l1s = wt("l1s"); v.tensor_add(l1s, s01, s23)
                pre = wt("pre")
                v.scalar_tensor_tensor(pre, l1s, 1.25, g,
                                       mybir.AluOpType.mult,
                                       mybir.AluOpType.subtract)
                ct = outp.tile([128, T], F32, tag="cost")
                v.tensor_mul(ct[0:nrow, :], pre, bc(11))

                nc.sync.dma_start(out=cost_d[k * 128:k * 128 + nrow, :],
                                  in_=ct[0:nrow, :])

    nc.finalize()
    return nc


def _host_prep(prob_class, predict_bbox, labels):
    """Build per-core input maps (all f32, mirroring reference's fp order)."""
    pc = np.ascontiguousarray(np.asarray(prob_class, np.float32))
    pb = np.asarray(predict_bbox, np.float32)
    lab = np.asarray(labels, np.float32)

    lb = lab[..., :4] / IMG_SIZE
    cls = lab[..., 4].astype(np.int32)
    mask = (cls != NO_OBJ).astype(np.float32)          # [B,T]

    lcx, lcy, lw, lh = (lb[..., i] for i in range(4))
    half = np.float32(0.5)
    lx0 = lcx - half * lw
    lx1 = lcx + half * lw
    ly0 = lcy - half * lh
    ly1 = lcy + half * lh
    lwE = lx1 - lx0
    lhE = ly1 - ly0
    areal = lwE * lhE

    quants = [lx0, ly0, lx1, ly1, lwE, lhE, lcx, lcy, lw, lh, areal, mask]
    qcat = np.zeros((B, NCHUNK, 512), np.float32)
    for q, arr in enumerate(quants):
        j, off = divmod(q, 3)
        qcat[:, j, off * 150:off * 150 + T] = arr

    # pred scalars [B, P, 12]
    pcx, pcy, pw, ph = (pb[..., i] for i in range(4))
    px0 = pcx - half * pw
    px1 = pcx + half * pw
    py0 = pcy - half * ph
    py1 = pcy + half * ph
    pwE = px1 - px0
    phE = py1 - py0
    apE = pwE * phE + EPS
    scal = np.zeros((B, P, 12), np.float32)
    for i, arr in enumerate([px0, py0, px1, py1, pwE, phE, apE,
                             -pcx, -pcy, -pw, -ph]):
        scal[:, :, i] = arr

    # -onehot + 4*mask rows [B, 128, T]
    rhsnm = np.zeros((B, 128, T), np.float32)
    bi = np.repeat(np.arange(B), T)
    ti = np.tile(np.arange(T), B)
    rhsnm[bi, cls.ravel(), ti] = np.float32(-1.0)
    rhsnm[:, 81, :] = np.float32(4.0) * mask

    # ln arg, transposed: [B, C, P]
    pt = np.ascontiguousarray(np.transpose(pc, (0, 2, 1)))

    # static tile tables
    sel = np.zeros((2, NTILES * 128), np.float32)
    qpair = np.zeros((NCORES, NTILES, 2, NCHUNK * 512), np.float32)
    for k, segs in enumerate(TILES):
        for si, (b, p0, ln, off) in enumerate(segs):
            sel[si, k * 128 + off:k * 128 + off + ln] = 1.0
            for core in range(NCORES):
                qpair[core, k, si] = qcat[core * BL + b].ravel()

    in_maps = []
    for core in range(NCORES):
        bsl = slice(core * BL, (core + 1) * BL)
        in_maps.append({
            "pt": pt[bsl],
            "rhsnm": rhsnm[bsl],
            "qpair": qpair[core],
            "sel": sel,
            "scal": scal[bsl].reshape(ROWS, 12),
        })
    return in_maps


def _hungarian_np(cost):
    """Jonker-Volgenant LSA fallback (same algorithm as scipy)."""
    cost = np.asarray(cost, dtype=np.float64)
    n, m = cost.shape
    INF = float("inf")
    u = np.zeros(n + 1)
    v = np.zeros(m + 1)
    p = np.zeros(m + 1, dtype=np.int64)
    way = np.zeros(m + 1, dtype=np.int64)
    for i in range(1, n + 1):
        p[0] = i
        j0 = 0
        minv = np.full(m + 1, INF)
        used = np.zeros(m + 1, dtype=bool)
        while True:
            used[j0] = True
            i0 = p[j0]
            free = ~used[1:]
            cur = cost[i0 - 1] - u[i0] - v[1:]
            better = free & (cur < minv[1:])
            minv[1:] = np.where(better, cur, minv[1:])
            way[1:] = np.where(better, j0, way[1:])
            masked = np.where(free, minv[1:], INF)
            j1 = int(np.argmin(masked)) + 1
            delta = masked[j1 - 1]
            uj = np.nonzero(used)[0]
            u[p[uj]] += delta
            v[uj] -= delta
            minv[1:] = np.where(free, minv[1:] - delta, minv[1:])
            j0 = j1
            if p[j0] == 0:
                break
        while j0 != 0:
            j1 = way[j0]
            p[j0] = p[j1]
            j0 = j1
    cols = np.nonzero(p[1:])[0]
    rows = p[1:][cols] - 1
    order = np.argsort(rows)
    return rows[order], cols[order]


def _lsa(cost):
    try:
        from scipy.optimize import linear_sum_assignment
        return linear_sum_assignment(cost)
    except Exception:
        return _hungarian_np(cost)


def _install_profile_shim():
    """Install the axon NTFF profile hook missing from this image's antenv,
    and neuter the (bucket-requiring) artifact upload."""
    import types

    if "antenv.axon_hooks" not in sys.modules:
        import antenv

        mod = types.ModuleType("antenv.axon_hooks")
        mod._HOOK = None
        mod.set_axon_ntff_profile_hook = lambda h: setattr(mod, "_HOOK", h)
        mod.get_axon_ntff_profile_hook = lambda: mod._HOOK
        sys.modules["antenv.axon_hooks"] = mod
        antenv.axon_hooks = mod
    import antenv.axon_hooks as ah

    if ah.get_axon_ntff_profile_hook() is None:
        try:
            from trn_agent_boot.trn_boot import _ntff_profile_via_ctypes

            hook = _ntff_profile_via_ctypes("/opt/axon/libaxon_pjrt.so")
            if hook is not None:
                ah.set_axon_ntff_profile_hook(hook)
        except Exception:
            pass
    import concourse.bass_utils as bu

    bu.upload_artifacts = lambda tmpdir: f"local:{tmpdir}"


def kernel(prob_class, predict_bbox, labels):
    global LAST_EXEC_NS, LAST_RESULTS
    if "nc" not in _CACHE:
        _CACHE["nc"] = _build_program()
    nc = _CACHE["nc"]

    in_maps = _host_prep(prob_class, predict_bbox, labels)
    if TRACE:
        _install_profile_shim()
    res = run_bass_kernel_spmd(nc, in_maps, list(range(NCORES)), trace=TRACE)
    LAST_EXEC_NS = res.exec_time_ns
    LAST_RESULTS = res

    cost = np.concatenate(
        [(res.results[i]["cost"] + res.results[i]["nllm"]).reshape(BL, P, T)
         for i in range(NCORES)], axis=0
    )

    loss = np.float32(0.0)
    cost64 = cost.astype(np.float64)
    for i in range(B):
        r, c = _lsa(cost64[i])
        loss = loss + cost[:, r, c].mean(dtype=np.float32)
    return np.float32(loss)


# revision 13
# speedup vs baseline: 1.1768x; 1.1768x over previous
"""DETR-style matching loss on 8 Trainium2 NeuronCores.

Device (data-parallel over batch, 8 samples/core): the [B,P,T] pairwise
cost matrix  cost = NLL + mask*(5*L1 + 2*GIoU_loss)  computed per core as
a [1200, 150] slab (rows = (b,p) packed, 10 tiles of 128 partitions).
Host: scipy Hungarian per sample on the device cost + the final
gather/mean (inherently sequential scalar work on 150x150 matrices).

v3 design:
- All per-(row,t) bilinear terms come from ONE bf16 3-pass PE matmul per
  PSUM bank (exact to ~2^-25): label-quantity broadcasts AND pairwise
  pre-sums (areal+apE, lwE+pwE, lhE+phE) AND pairwise pre-diffs for L1
  (lcx-pcx, ...). K=27: 6 selector rows (2 segments x hi/mid/lo) + 7
  pred-side values x hi/mid/lo against slot-indicator rows.
- NLL + 4*mask from an fp32 one-hot matmul (exact: one product/output).
- The nonlinear chain runs as 8 fused custom-DVE ops + a few stock
  tensor ops balanced across DVE/GpSimd/ScalarE.

Math (per pair, r=(b,p) row, t):
  iw = min(px1,lx1) - max(px0,lx0);  ih likewise;  inter = relu.iw*relu.ih
  ue = (areal + area_p + EPS) - inter          [= union + EPS]
  ew = (lwE + pwE) - iw;  eh = (lhE + phE) - ih   [min+max=sum identity]
  ee = ew*eh + EPS                              [= enclose + EPS]
  rc = 1/(ue*ee)   [approx recip + 1 Newton step]
  cost = [nll + 4*mask] + 2*mask*( 0.625*sum|d| - (inter*ee + ue^2)*rc )
"""

import sys

import numpy as np

for _p in ("/opt/trn_rl_repo",):
    if _p not in sys.path:
        sys.path.append(_p)

import ml_dtypes

import concourse.bacc as bacc
import concourse.tile as tile
from concourse import mybir
from concourse.bass_utils import run_bass_kernel_spmd

F32 = mybir.dt.float32
BF16 = mybir.dt.bfloat16
NPBF = ml_dtypes.bfloat16
EPS = np.float32(1e-7)
IMG_SIZE = np.float32(320.0)
NO_OBJ = 80

B, P, T, C = 64, 150, 150, 81
NCORES = 8
BL = B // NCORES          # 8 samples per core
ROWS = BL * P             # 1200 (b,p) rows per core
NTILES = (ROWS + 127) // 128   # 10
NK = 27                   # bc-matmul contraction rows

# bc slot layout: slot q at psum cols (q//3)*512 + (q%3)*150.
# chunk0 (slots 0-2) stays PSUM-resident; chunks 1-3 are copied to SBUF.
SLOT = {"lx1": 0, "ly1": 1, "dcx": 2,
        "lx0": 3, "ly0": 4, "dcy": 5,
        "dw": 6, "dh": 7, "mask2": 8,
        "uepre": 9, "ewpre": 10, "ehpre": 11}
# pred-side presum value index -> target slot (7 presums, rows 6+3i..8+3i)
PRESUMS = ["dcx", "dcy", "dw", "dh", "uepre", "ewpre", "ehpre"]


def _tile_segments():
    tiles = []
    for k in range(NTILES):
        r0, r1 = k * 128, min(k * 128 + 128, ROWS)
        segs, r = [], r0
        while r < r1:
            b = r // P
            p0 = r % P
            ln = min(r1 - r, P - p0)
            segs.append((b, p0, ln, r - r0))
            r += ln
        tiles.append(segs)
    return tiles


TILES = _tile_segments()

_CACHE = {}
TRACE = False
DEBUG = False
LAST_EXEC_NS = None
LAST_RESULTS = None


def _register_custom_ops():
    if "ops" in _CACHE:
        return _CACHE["ops"]
    from concourse import dve_ops
    from concourse.dve_spec import (
        Spec, Src0, Src1, C0, C1, C2, Zero, relu, sq, maxx, minn, lower,
        _has_src1,
    )
    from concourse.dve_uop import DveOpSpec
    from concourse.dve_table_gen import dve_ver_for

    def ref_iw(in0, in1, s0, s1, imm2):
        return np.minimum(in0, s0) - np.maximum(in1, s1)

    def ref_relumul(in0, in1, s0, s1, imm2):
        return np.maximum(in0, 0) * np.maximum(in1, 0)

    def ref_fmaeps(in0, in1, s0, s1, imm2):
        return in0 * in1 + imm2

    def ref_addsq(in0, in1, s0, s1, imm2):
        return in0 + in1 * in1

    def ref_abs2(in0, in1, s0, s1, imm2):
        return (np.abs(in0) + np.abs(in1)) * imm2

    specs = {
        "DETR_IW": Spec(body=minn(Src0, C0) - maxx(Src1, C1),
                        reference=ref_iw),
        "DETR_RELUMUL": Spec(body=relu(Src0) * relu(Src1),
                             reference=ref_relumul),
        "DETR_FMAEPS": Spec(body=Src0 * Src1 + C2, reference=ref_fmaeps),
        "DETR_ADDSQ": Spec(body=Src0 + sq(Src1), reference=ref_addsq),
        "DETR_ABS2": Spec(
            body=(maxx(Src0, Zero - Src0) + maxx(Src1, Zero - Src1)) * C2,
            reference=ref_abs2),
    }
    ops = {}
    existing = {op.name for op in dve_ops.OPS}
    for name, spec in specs.items():
        if name in existing:
            ops[name] = next(o for o in dve_ops.OPS if o.name == name)
            continue
        shas = {}
        for ver in ("v3", "v4"):
            try:
                uops = lower(spec, ver=ver)
                s = DveOpSpec(name=name, opcode=1, uops=uops,
                              rd1_en=_has_src1(spec))
                shas[ver] = s.sha(ver)
            except Exception:
                pass
        op = dve_ops.DveOp(name, spec, subdim=False, uops_sha=shas)
        dve_ops.OPS.append(op)
        dve_ops.CUSTOM_DVE_SPECS[name] = spec
        dve_ops._SUB_OPCODE_FOR_NAME[name] = (
            max(dve_ops._SUB_OPCODE_FOR_NAME.values()) + 1)
        ops[name] = op
    ops["FAST"] = dve_ops.RECIPROCAL_APPROX_FAST
    ops["NR"] = dve_ops.RECIPROCAL_APPROX_NR
    _CACHE["ops"] = ops
    return ops


def _build_program():
    ops = _register_custom_ops()
    nc = bacc.Bacc(None, target_bir_lowering=False)

    pt_d = nc.declare_dram_parameter("pt", [BL, C, P], F32, isOutput=False)
    rhs_d = nc.declare_dram_parameter("rhsnm", [BL, 128, T], F32,
                                      isOutput=False)
    bcl_d = nc.declare_dram_parameter("bcl", [NTILES, NK, 128], BF16,
                                      isOutput=False)
    bcr_d = nc.declare_dram_parameter("bcr", [NTILES, NK, 2048], BF16,
                                      isOutput=False)
    scal_d = nc.declare_dram_parameter("scal", [ROWS, 4], F32, isOutput=False)
    cost_d = nc.declare_dram_parameter("cost", [ROWS, T], F32, isOutput=True)
    nll_d = nc.declare_dram_parameter("nllm", [ROWS, T], F32, isOutput=True)
    if DEBUG:
        dbg_d = nc.declare_dram_parameter("dbg", [ROWS, 2048], F32,
                                          isOutput=True)
        dbg2_d = nc.declare_dram_parameter("dbg2", [ROWS, 3 * T], F32,
                                           isOutput=True)

    from contextlib import ExitStack

    with tile.TileContext(nc) as tc, ExitStack() as ctx:
        perb = ctx.enter_context(tc.tile_pool(name="perb", bufs=3))
        ptp = ctx.enter_context(tc.tile_pool(name="ptp", bufs=2))
        pert = ctx.enter_context(tc.tile_pool(name="pert", bufs=3))
        bcs = ctx.enter_context(tc.tile_pool(name="bcs", bufs=2))
        work = ctx.enter_context(tc.tile_pool(name="work", bufs=2))
        outp = ctx.enter_context(tc.tile_pool(name="outp", bufs=3))
        bcp = ctx.enter_context(tc.tile_pool(name="bcp", bufs=1, space="PSUM"))
        nllp = ctx.enter_context(tc.tile_pool(name="nllp", bufs=2,
                                              space="PSUM"))

        lhs = {}
        rhs = {}

        def ensure_b(b):
            if b in lhs:
                return
            ptile = ptp.tile([C, P], F32, tag="ptile")
            nc.sync.dma_start(out=ptile[:], in_=pt_d[b])
            lt = perb.tile([128, P], F32, tag="lhs")
            nc.vector.memset(lt[64:128, :], 1.0)
            nc.scalar.activation(out=lt[0:C, :], in_=ptile[:],
                                 func=mybir.ActivationFunctionType.Ln)
            rt = perb.tile([128, T], F32, tag="rhs")
            nc.sync.dma_start(out=rt[:], in_=rhs_d[b])
            lhs[b] = lt
            rhs[b] = rt

        v = nc.vector
        g = nc.gpsimd
        s = nc.scalar

        for k, segs in enumerate(TILES):
            nrow = sum(sg[2] for sg in segs)
            for b, _, _, _ in segs:
                ensure_b(b)

            # ---- bc matmuls: one per PSUM bank, K=27 bf16 ----
            lt27 = pert.tile([NK, 128], BF16, tag="lt27")
            nc.sync.dma_start(out=lt27[:], in_=bcl_d[k])
            rt27 = pert.tile([NK, 2048], BF16, tag="rt27")
            nc.sync.dma_start(out=rt27[:], in_=bcr_d[k])
            ps = bcp.tile([128, 2048], F32, tag="bcps")
            for j in range(4):
                nc.tensor.matmul(ps[:, j * 512:(j + 1) * 512], lt27[:, :],
                                 rt27[:, j * 512:(j + 1) * 512],
                                 start=True, stop=True)

            # copy chunks 1-3 to SBUF (split DVE / ACT)
            sb = bcs.tile([128, 2048], F32, tag="bcsb")
            v.tensor_copy(sb[:, 512:1024], ps[:, 512:1024])
            s.copy(sb[:, 1024:2048], ps[:, 1024:2048])

            def bcP(name):
                q = SLOT[name]
                off = (q // 3) * 512 + (q % 3) * 150
                return ps[0:nrow, off:off + T]

            def bcS(name):
                q = SLOT[name]
                off = (q // 3) * 512 + (q % 3) * 150
                return sb[0:nrow, off:off + T]

            # ---- nll + 4*mask (fp32 one-hot matmul per segment) ----
            for si, (b, p0, ln, off) in enumerate(segs):
                nm = nllp.tile([128, 512], F32, tag=f"nll{si}",
                               name=f"nll{si}")
                nc.tensor.matmul(nm[0:ln, 0:T], lhs[b][:, p0:p0 + ln],
                                 rhs[b][:, :], start=True, stop=True)
                nsb = outp.tile([128, T], F32, tag=f"nsb{si}",
                                name=f"nsb{si}")
                s.copy(out=nsb[0:ln, :], in_=nm[0:ln, 0:T])
                nc.sync.dma_start(
                    out=nll_d[k * 128 + off:k * 128 + off + ln, :],
                    in_=nsb[0:ln, :])

            # ---- pred xyxy scalars ----
            sc = pert.tile([128, 4], F32, tag="scal")
            nc.sync.dma_start(out=sc[0:nrow, :],
                              in_=scal_d[k * 128:k * 128 + nrow, :])
            px1, px0, py1, py0 = (sc[0:nrow, i:i + 1] for i in range(4))

            def wt(tag, w=T):
                return work.tile([128, w], F32, tag=tag, name=tag)

            # ---- fused chain ----
            # t3 = [inter | iw | ih] adjacent for the GPS triple-subtract
            t3 = wt("t3", 3 * T)
            iw = t3[0:nrow, T:2 * T]
            ih = t3[0:nrow, 2 * T:3 * T]
            inter = t3[0:nrow, 0:T]
            v._custom_dve(ops["DETR_IW"], out=iw, in0=bcP("lx1"),
                          in1=bcS("lx0"), s0=px1, s1=px0)
            v._custom_dve(ops["DETR_IW"], out=ih, in0=bcP("ly1"),
                          in1=bcS("ly0"), s0=py1, s1=py0)
            v._custom_dve(ops["DETR_RELUMUL"], out=inter, in0=iw, in1=ih)

            # [ue|ew|eh] = [uepre|ewpre|ehpre] - [inter|iw|ih]  (one GPS op)
            uew = wt("uew", 3 * T)
            g.tensor_sub(uew[0:nrow, :], sb[0:nrow, 1536:1536 + 3 * T],
                         t3[0:nrow, :])
            ue = uew[0:nrow, 0:T]
            ew = uew[0:nrow, T:2 * T]
            eh = uew[0:nrow, 2 * T:3 * T]

            ee = wt("ee")[0:nrow, :]
            v._custom_dve(ops["DETR_FMAEPS"], out=ee, in0=ew, in1=eh,
                          imm2=float(EPS))
            prod = wt("prod")[0:nrow, :]
            v.tensor_mul(prod, ue, ee)
            t1 = wt("t1")[0:nrow, :]
            v.tensor_mul(t1, inter, ee)
            r0 = wt("r0")[0:nrow, :]
            rc = wt("rc")[0:nrow, :]
            v.reciprocal_approx_accurate(out=rc, in_=prod, scratch=r0)
            gnum = wt("gnum")[0:nrow, :]
            v._custom_dve(ops["DETR_ADDSQ"], out=gnum, in0=t1, in1=ue)
            gterm = wt("gterm")[0:nrow, :]
            g.tensor_mul(gterm, gnum, rc)

            s01 = wt("s01")[0:nrow, :]
            v._custom_dve(ops["DETR_ABS2"], out=s01, in0=bcP("dcx"),
                          in1=bcS("dcy"), imm2=0.625)
            s23 = wt("s23")[0:nrow, :]
            v._custom_dve(ops["DETR_ABS2"], out=s23, in0=bcS("dw"),
                          in1=bcS("dh"), imm2=0.625)
            l1t = wt("l1t")[0:nrow, :]
            g.tensor_add(l1t, s01, s23)
            pre = wt("pre")[0:nrow, :]
            g.tensor_sub(pre, l1t, gterm)
            mp = outp.tile([128, T], F32, tag="mp")
            g.tensor_mul(mp[0:nrow, :], pre, bcS("mask2"))

            nc.sync.dma_start(out=cost_d[k * 128:k * 128 + nrow, :],
                              in_=mp[0:nrow, :])
            if DEBUG:
                dbt = outp.tile([128, 512], F32, tag="dbt")
                s.copy(out=dbt[0:nrow, :], in_=ps[0:nrow, 0:512])
                nc.sync.dma_start(
                    out=dbg_d[k * 128:k * 128 + nrow, 0:512],
                    in_=dbt[0:nrow, :])
                nc.sync.dma_start(
                    out=dbg_d[k * 128:k * 128 + nrow, 512:2048],
                    in_=sb[0:nrow, 512:2048])
                db3 = outp.tile([128, 3 * T], F32, tag="db3")
                s.copy(out=db3[0:nrow, 0:T], in_=ee)
                s.copy(out=db3[0:nrow, T:2 * T], in_=rc)
                s.copy(out=db3[0:nrow, 2 * T:3 * T], in_=gterm)
                nc.sync.dma_start(
                    out=dbg2_d[k * 128:k * 128 + nrow, :],
                    in_=db3[0:nrow, :])

    nc.finalize()
    return nc


def _decomp3(v):
    """f32 -> three bf16 arrays summing to v (error ~2^-25 |v|)."""
    v = np.asarray(v, np.float32)
    h = v.astype(NPBF)
    r = v - h.astype(np.float32)
    m = r.astype(NPBF)
    lo = (r - m.astype(np.float32)).astype(NPBF)
    return h, m, lo


def _host_prep(prob_class, predict_bbox, labels):
    pc = np.ascontiguousarray(np.asarray(prob_class, np.float32))
    pb = np.asarray(predict_bbox, np.float32)
    lab = np.asarray(labels, np.float32)

    lb = lab[..., :4] / IMG_SIZE
    cls = lab[..., 4].astype(np.int32)
    mask = (cls != NO_OBJ).astype(np.float32)

    lcx, lcy, lw, lh = (lb[..., i] for i in range(4))
    half = np.float32(0.5)
    lx0 = lcx - half * lw
    lx1 = lcx + half * lw
    ly0 = lcy - half * lh
    ly1 = lcy + half * lh
    lwE = lx1 - lx0
    lhE = ly1 - ly0
    areal = lwE * lhE

    labvals = {"lx1": lx1, "ly1": ly1, "dcx": lcx,
               "lx0": lx0, "ly0": ly0, "dcy": lcy,
               "dw": lw, "dh": lh, "mask2": 2.0 * mask,
               "uepre": areal, "ewpre": lwE, "ehpre": lhE}
    qlab = np.zeros((B, 2048), np.float32)
    for name, q in SLOT.items():
        off = (q // 3) * 512 + (q % 3) * 150
        qlab[:, off:off + T] = labvals[name]

    pcx, pcy, pw, ph = (pb[..., i] for i in range(4))
    px0 = pcx - half * pw
    px1 = pcx + half * pw
    py0 = pcy - half * ph
    py1 = pcy + half * ph
    pwE = px1 - px0
    phE = py1 - py0
    apE = pwE * phE + EPS
    presvals = {"dcx": -pcx, "dcy": -pcy, "dw": -pw, "dh": -ph,
                "uepre": apE, "ewpre": pwE, "ehpre": phE}   # [B, P] each

    scal = np.stack([px1, px0, py1, py0], axis=-1)  # [B, P, 4]

    rhsnm = np.zeros((B, 128, T), np.float32)
    bi = np.repeat(np.arange(B), T)
    ti = np.tile(np.arange(T), B)
    rhsnm[bi, cls.ravel(), ti] = np.float32(-1.0)
    rhsnm[:, 81, :] = np.float32(4.0) * mask

    pt = np.ascontiguousarray(np.transpose(pc, (0, 2, 1)))  # [B, C, P]

    qlab3 = _decomp3(qlab)                       # 3 x [B, 2048] bf16
    pres3 = {n: _decomp3(val) for n, val in presvals.items()}

    bcl = np.zeros((NCORES, NTILES, NK, 128), NPBF)
    bcr = np.zeros((NCORES, NTILES, NK, 2048), NPBF)
    for k, segs in enumerate(TILES):
        for core in range(NCORES):
            for si, (b, p0, ln, off) in enumerate(segs):
                gb = core * BL + b
                for p3 in range(3):
                    bcl[core, k, 3 * si + p3, off:off + ln] = NPBF(1.0)
                    bcr[core, k, 3 * si + p3, :] = qlab3[p3][gb]
                for i, name in enumerate(PRESUMS):
                    q = SLOT[name]
                    coff = (q // 3) * 512 + (q % 3) * 150
                    for p3 in range(3):
                        bcl[core, k, 6 + 3 * i + p3, off:off + ln] = \
                            pres3[name][p3][gb, p0:p0 + ln]
                        bcr[core, k, 6 + 3 * i + p3, coff:coff + T] = NPBF(1.0)

    in_maps = []
    for core in range(NCORES):
        bsl = slice(core * BL, (core + 1) * BL)
        in_maps.append({
            "pt": pt[bsl],
            "rhsnm": rhsnm[bsl],
            "bcl": bcl[core],
            "bcr": bcr[core],
            "scal": scal[bsl].reshape(ROWS, 4),
        })
    return in_maps


def _hungarian_np(cost):
    """Jonker-Volgenant LSA fallback (same algorithm as scipy)."""
    cost = np.asarray(cost, dtype=np.float64)
    n, m = cost.shape
    INF = float("inf")
    u = np.zeros(n + 1)
    vv = np.zeros(m + 1)
    p = np.zeros(m + 1, dtype=np.int64)
    way = np.zeros(m + 1, dtype=np.int64)
    for i in range(1, n + 1):
        p[0] = i
        j0 = 0
        minv = np.full(m + 1, INF)
        used = np.zeros(m + 1, dtype=bool)
        while True:
            used[j0] = True
            i0 = p[j0]
            free = ~used[1:]
            cur = cost[i0 - 1] - u[i0] - vv[1:]
            better = free & (cur < minv[1:])
            minv[1:] = np.where(better, cur, minv[1:])
            way[1:] = np.where(better, j0, way[1:])
            masked = np.where(free, minv[1:], INF)
            j1 = int(np.argmin(masked)) + 1
            delta = masked[j1 - 1]
            uj = np.nonzero(used)[0]
            u[p[uj]] += delta
            vv[uj] -= delta
            minv[1:] = np.where(free, minv[1:] - delta, minv[1:])
            j0 = j1
            if p[j0] == 0:
                break
        while j0 != 0:
            j1 = way[j0]
            p[j0] = p[j1]
            j0 = j1
    cols = np.nonzero(p[1:])[0]
    rows = p[1:][cols] - 1
    order = np.argsort(rows)
    return rows[order], cols[order]


def _lsa(cost):
    try:
        from scipy.optimize import linear_sum_assignment
        return linear_sum_assignment(cost)
    except Exception:
        return _hungarian_np(cost)


def _install_profile_shim():
    import types

    if "antenv.axon_hooks" not in sys.modules:
        import antenv

        mod = types.ModuleType("antenv.axon_hooks")
        mod._HOOK = None
        mod.set_axon_ntff_profile_hook = lambda h: setattr(mod, "_HOOK", h)
        mod.get_axon_ntff_profile_hook = lambda: mod._HOOK
        sys.modules["antenv.axon_hooks"] = mod
        antenv.axon_hooks = mod
    import antenv.axon_hooks as ah

    if ah.get_axon_ntff_profile_hook() is None:
        try:
            from trn_agent_boot.trn_boot import _ntff_profile_via_ctypes

            hook = _ntff_profile_via_ctypes("/opt/axon/libaxon_pjrt.so")
            if hook is not None:
                ah.set_axon_ntff_profile_hook(hook)
        except Exception:
            pass
    import concourse.bass_utils as bu

    bu.upload_artifacts = lambda tmpdir: f"local:{tmpdir}"


def kernel(prob_class, predict_bbox, labels):
    global LAST_EXEC_NS, LAST_RESULTS
    if "nc" not in _CACHE:
        _CACHE["nc"] = _build_program()
    nc = _CACHE["nc"]

    in_maps = _host_prep(prob_class, predict_bbox, labels)
    if TRACE:
        _install_profile_shim()
    res = run_bass_kernel_spmd(nc, in_maps, list(range(NCORES)), trace=TRACE)
    LAST_EXEC_NS = res.exec_time_ns
    LAST_RESULTS = res

    cost = np.concatenate(
        [(res.results[i]["cost"] + res.results[i]["nllm"]).reshape(BL, P, T)
         for i in range(NCORES)], axis=0
    )

    loss = np.float32(0.0)
    cost64 = cost.astype(np.float64)
    for i in range(B):
        r, c = _lsa(cost64[i])
        loss = loss + cost[:, r, c].mean(dtype=np.float32)
    return np.float32(loss)


# revision 15
# speedup vs baseline: 1.1877x; 1.0092x over previous
"""DETR-style matching loss on 8 Trainium2 NeuronCores.

Device (data-parallel over batch, 8 samples/core): the [B,P,T] pairwise
cost matrix  cost = NLL + mask*(5*L1 + 2*GIoU_loss)  computed per core as
a [1200, 150] slab (rows = (b,p) packed, 10 tiles of 128 partitions).
Host: scipy Hungarian per sample on the device cost + the final
gather/mean (inherently sequential scalar work on 150x150 matrices).

v3 design:
- All per-(row,t) bilinear terms come from ONE bf16 3-pass PE matmul per
  PSUM bank (exact to ~2^-25): label-quantity broadcasts AND pairwise
  pre-sums (areal+apE, lwE+pwE, lhE+phE) AND pairwise pre-diffs for L1
  (lcx-pcx, ...). K=27: 6 selector rows (2 segments x hi/mid/lo) + 7
  pred-side values x hi/mid/lo against slot-indicator rows.
- NLL + 4*mask from an fp32 one-hot matmul (exact: one product/output).
- The nonlinear chain runs as 8 fused custom-DVE ops + a few stock
  tensor ops balanced across DVE/GpSimd/ScalarE.

Math (per pair, r=(b,p) row, t):
  iw = min(px1,lx1) - max(px0,lx0);  ih likewise;  inter = relu.iw*relu.ih
  ue = (areal + area_p + EPS) - inter          [= union + EPS]
  ew = (lwE + pwE) - iw;  eh = (lhE + phE) - ih   [min+max=sum identity]
  ee = ew*eh + EPS                              [= enclose + EPS]
  rc = 1/(ue*ee)   [approx recip + 1 Newton step]
  cost = [nll + 4*mask] + 2*mask*( 0.625*sum|d| - (inter*ee + ue^2)*rc )
"""

import sys

import numpy as np

for _p in ("/opt/trn_rl_repo",):
    if _p not in sys.path:
        sys.path.append(_p)

import ml_dtypes

import concourse.bacc as bacc
import concourse.tile as tile
from concourse import mybir
from concourse.bass_utils import run_bass_kernel_spmd

F32 = mybir.dt.float32
BF16 = mybir.dt.bfloat16
NPBF = ml_dtypes.bfloat16
EPS = np.float32(1e-7)
IMG_SIZE = np.float32(320.0)
NO_OBJ = 80

B, P, T, C = 64, 150, 150, 81
NCORES = 8
BL = B // NCORES          # 8 samples per core
ROWS = BL * P             # 1200 (b,p) rows per core
NTILES = (ROWS + 127) // 128   # 10
NK = 27                   # bc-matmul contraction rows

# bc slot layout: slot q at psum cols (q//3)*512 + (q%3)*150.
# chunk0 (slots 0-2) stays PSUM-resident; chunks 1-3 are copied to SBUF.
SLOT = {"lx1": 0, "ly1": 1, "dcx": 2,
        "lx0": 3, "ly0": 4, "dcy": 5,
        "dw": 6, "dh": 7, "mask2": 8,
        "uepre": 9, "ewpre": 10, "ehpre": 11}
# pred-side presum value index -> target slot (7 presums, rows 6+3i..8+3i)
PRESUMS = ["dcx", "dcy", "dw", "dh", "uepre", "ewpre", "ehpre"]


def _tile_segments():
    tiles = []
    for k in range(NTILES):
        r0, r1 = k * 128, min(k * 128 + 128, ROWS)
        segs, r = [], r0
        while r < r1:
            b = r // P
            p0 = r % P
            ln = min(r1 - r, P - p0)
            segs.append((b, p0, ln, r - r0))
            r += ln
        tiles.append(segs)
    return tiles


TILES = _tile_segments()

_CACHE = {}
TRACE = False
DEBUG = False
LAST_EXEC_NS = None
LAST_RESULTS = None


def _register_custom_ops():
    if "ops" in _CACHE:
        return _CACHE["ops"]
    from concourse import dve_ops
    from concourse.dve_spec import (
        Spec, Src0, Src1, C0, C1, C2, Zero, relu, sq, maxx, minn, lower,
        _has_src1,
    )
    from concourse.dve_uop import DveOpSpec
    from concourse.dve_table_gen import dve_ver_for

    def ref_iw(in0, in1, s0, s1, imm2):
        return np.minimum(in0, s0) - np.maximum(in1, s1)

    def ref_relumul(in0, in1, s0, s1, imm2):
        return np.maximum(in0, 0) * np.maximum(in1, 0)

    def ref_fmaeps(in0, in1, s0, s1, imm2):
        return in0 * in1 + imm2

    def ref_addsq(in0, in1, s0, s1, imm2):
        return in0 + in1 * in1

    def ref_abs2(in0, in1, s0, s1, imm2):
        return (np.abs(in0) + np.abs(in1)) * imm2

    specs = {
        "DETR_IW": Spec(body=minn(Src0, C0) - maxx(Src1, C1),
                        reference=ref_iw),
        "DETR_RELUMUL": Spec(body=relu(Src0) * relu(Src1),
                             reference=ref_relumul),
        "DETR_FMAEPS": Spec(body=Src0 * Src1 + C2, reference=ref_fmaeps),
        "DETR_ADDSQ": Spec(body=Src0 + sq(Src1), reference=ref_addsq),
        "DETR_ABS2": Spec(
            body=(maxx(Src0, Zero - Src0) + maxx(Src1, Zero - Src1)) * C2,
            reference=ref_abs2),
    }
    ops = {}
    existing = {op.name for op in dve_ops.OPS}
    for name, spec in specs.items():
        if name in existing:
            ops[name] = next(o for o in dve_ops.OPS if o.name == name)
            continue
        shas = {}
        for ver in ("v3", "v4"):
            try:
                uops = lower(spec, ver=ver)
                s = DveOpSpec(name=name, opcode=1, uops=uops,
                              rd1_en=_has_src1(spec))
                shas[ver] = s.sha(ver)
            except Exception:
                pass
        op = dve_ops.DveOp(name, spec, subdim=False, uops_sha=shas)
        dve_ops.OPS.append(op)
        dve_ops.CUSTOM_DVE_SPECS[name] = spec
        dve_ops._SUB_OPCODE_FOR_NAME[name] = (
            max(dve_ops._SUB_OPCODE_FOR_NAME.values()) + 1)
        ops[name] = op
    ops["FAST"] = dve_ops.RECIPROCAL_APPROX_FAST
    ops["NR"] = dve_ops.RECIPROCAL_APPROX_NR
    _CACHE["ops"] = ops
    return ops


def _build_program():
    ops = _register_custom_ops()
    nc = bacc.Bacc(None, target_bir_lowering=False)

    pt_d = nc.declare_dram_parameter("pt", [BL, C, P], F32, isOutput=False)
    rhs_d = nc.declare_dram_parameter("rhsnm", [BL, 128, T], F32,
                                      isOutput=False)
    bcr_d = nc.declare_dram_parameter("bcr", [NTILES, NK, 2176], BF16,
                                      isOutput=False)
    scal_d = nc.declare_dram_parameter("scal", [ROWS, 4], F32, isOutput=False)
    cost_d = nc.declare_dram_parameter("cost", [ROWS, T], F32, isOutput=True)
    nll_d = nc.declare_dram_parameter("nllm", [ROWS, T], F32, isOutput=True)
    if DEBUG:
        dbg_d = nc.declare_dram_parameter("dbg", [ROWS, 2048], F32,
                                          isOutput=True)
        dbg2_d = nc.declare_dram_parameter("dbg2", [ROWS, 3 * T], F32,
                                           isOutput=True)

    from contextlib import ExitStack

    with tile.TileContext(nc) as tc, ExitStack() as ctx:
        perb = ctx.enter_context(tc.tile_pool(name="perb", bufs=3))
        ptp = ctx.enter_context(tc.tile_pool(name="ptp", bufs=2))
        pert = ctx.enter_context(tc.tile_pool(name="pert", bufs=3))
        bcs = ctx.enter_context(tc.tile_pool(name="bcs", bufs=3))
        work = ctx.enter_context(tc.tile_pool(name="work", bufs=3))
        outp = ctx.enter_context(tc.tile_pool(name="outp", bufs=4))
        bcp = ctx.enter_context(tc.tile_pool(name="bcp", bufs=1, space="PSUM"))
        nllp = ctx.enter_context(tc.tile_pool(name="nllp", bufs=2,
                                              space="PSUM"))

        lhs = {}
        rhs = {}
        qrr = [0]

        def dma(out, in_):
            qrr[0] ^= 1
            eng = nc.sync if qrr[0] else nc.scalar
            eng.dma_start(out=out, in_=in_)

        def ensure_b(b):
            if b in lhs:
                return
            ptile = ptp.tile([C, P], F32, tag="ptile")
            dma(ptile[:], pt_d[b])
            lt = perb.tile([128, P], F32, tag="lhs")
            nc.vector.memset(lt[64:128, :], 1.0)
            nc.scalar.activation(out=lt[0:C, :], in_=ptile[:],
                                 func=mybir.ActivationFunctionType.Ln)
            rt = perb.tile([128, T], F32, tag="rhs")
            dma(rt[:], rhs_d[b])
            lhs[b] = lt
            rhs[b] = rt

        v = nc.vector
        g = nc.gpsimd
        s = nc.scalar

        for k, segs in enumerate(TILES):
            nrow = sum(sg[2] for sg in segs)
            for b, _, _, _ in segs:
                ensure_b(b)

            # ---- bc matmuls: one per PSUM bank, K=27 bf16 ----
            rt27 = pert.tile([NK, 2176], BF16, tag="rt27")
            dma(rt27[:], bcr_d[k])
            lt27 = rt27[:, 2048:2176]
            ps = bcp.tile([128, 2048], F32, tag="bcps")
            for j in range(4):
                nc.tensor.matmul(ps[:, j * 512:(j + 1) * 512], lt27,
                                 rt27[:, j * 512:(j + 1) * 512],
                                 start=True, stop=True)

            # copy chunks 1-3 to SBUF (split DVE / ACT)
            sb = bcs.tile([128, 2048], F32, tag="bcsb")
            v.tensor_copy(sb[:, 512:1024], ps[:, 512:1024])
            s.copy(sb[:, 1024:2048], ps[:, 1024:2048])

            def bcP(name):
                q = SLOT[name]
                off = (q // 3) * 512 + (q % 3) * 150
                return ps[0:nrow, off:off + T]

            def bcS(name):
                q = SLOT[name]
                off = (q // 3) * 512 + (q % 3) * 150
                return sb[0:nrow, off:off + T]

            # ---- nll + 4*mask (fp32 one-hot matmul per segment) ----
            for si, (b, p0, ln, off) in enumerate(segs):
                nm = nllp.tile([128, 512], F32, tag=f"nll{si}",
                               name=f"nll{si}")
                nc.tensor.matmul(nm[0:ln, 0:T], lhs[b][:, p0:p0 + ln],
                                 rhs[b][:, :], start=True, stop=True)
                nsb = outp.tile([128, T], F32, tag=f"nsb{si}",
                                name=f"nsb{si}")
                s.copy(out=nsb[0:ln, :], in_=nm[0:ln, 0:T])
                dma(nll_d[k * 128 + off:k * 128 + off + ln, :],
                    nsb[0:ln, :])

            # ---- pred xyxy scalars ----
            sc = pert.tile([128, 4], F32, tag="scal")
            dma(sc[0:nrow, :], scal_d[k * 128:k * 128 + nrow, :])
            px1, px0, py1, py0 = (sc[0:nrow, i:i + 1] for i in range(4))

            def wt(tag, w=T):
                return work.tile([128, w], F32, tag=tag, name=tag)

            # ---- fused chain ----
            # t3 = [inter | iw | ih] adjacent for the GPS triple-subtract
            t3 = wt("t3", 3 * T)
            iw = t3[0:nrow, T:2 * T]
            ih = t3[0:nrow, 2 * T:3 * T]
            inter = t3[0:nrow, 0:T]
            v._custom_dve(ops["DETR_IW"], out=iw, in0=bcP("lx1"),
                          in1=bcS("lx0"), s0=px1, s1=px0)
            v._custom_dve(ops["DETR_IW"], out=ih, in0=bcP("ly1"),
                          in1=bcS("ly0"), s0=py1, s1=py0)
            v._custom_dve(ops["DETR_RELUMUL"], out=inter, in0=iw, in1=ih)

            # [ue|ew|eh] = [uepre|ewpre|ehpre] - [inter|iw|ih]  (one GPS op)
            uew = wt("uew", 3 * T)
            g.tensor_sub(uew[0:nrow, :], sb[0:nrow, 1536:1536 + 3 * T],
                         t3[0:nrow, :])
            ue = uew[0:nrow, 0:T]
            ew = uew[0:nrow, T:2 * T]
            eh = uew[0:nrow, 2 * T:3 * T]

            ee = wt("ee")[0:nrow, :]
            v._custom_dve(ops["DETR_FMAEPS"], out=ee, in0=ew, in1=eh,
                          imm2=float(EPS))
            prod = wt("prod")[0:nrow, :]
            v.tensor_mul(prod, ue, ee)
            t1 = wt("t1")[0:nrow, :]
            v.tensor_mul(t1, inter, ee)
            r0 = wt("r0")[0:nrow, :]
            rc = wt("rc")[0:nrow, :]
            v.reciprocal_approx_accurate(out=rc, in_=prod, scratch=r0)
            gnum = wt("gnum")[0:nrow, :]
            v._custom_dve(ops["DETR_ADDSQ"], out=gnum, in0=t1, in1=ue)
            gterm = wt("gterm")[0:nrow, :]
            g.tensor_mul(gterm, gnum, rc)

            s01 = wt("s01")[0:nrow, :]
            v._custom_dve(ops["DETR_ABS2"], out=s01, in0=bcP("dcx"),
                          in1=bcS("dcy"), imm2=0.625)
            s23 = wt("s23")[0:nrow, :]
            v._custom_dve(ops["DETR_ABS2"], out=s23, in0=bcS("dw"),
                          in1=bcS("dh"), imm2=0.625)
            l1t = wt("l1t")[0:nrow, :]
            g.tensor_add(l1t, s01, s23)
            pre = wt("pre")[0:nrow, :]
            g.tensor_sub(pre, l1t, gterm)
            mp = outp.tile([128, T], F32, tag="mp")
            g.tensor_mul(mp[0:nrow, :], pre, bcS("mask2"))

            dma(cost_d[k * 128:k * 128 + nrow, :], mp[0:nrow, :])
            if DEBUG:
                dbt = outp.tile([128, 512], F32, tag="dbt")
                s.copy(out=dbt[0:nrow, :], in_=ps[0:nrow, 0:512])
                nc.sync.dma_start(
                    out=dbg_d[k * 128:k * 128 + nrow, 0:512],
                    in_=dbt[0:nrow, :])
                nc.sync.dma_start(
                    out=dbg_d[k * 128:k * 128 + nrow, 512:2048],
                    in_=sb[0:nrow, 512:2048])
                db3 = outp.tile([128, 3 * T], F32, tag="db3")
                s.copy(out=db3[0:nrow, 0:T], in_=ee)
                s.copy(out=db3[0:nrow, T:2 * T], in_=rc)
                s.copy(out=db3[0:nrow, 2 * T:3 * T], in_=gterm)
                nc.sync.dma_start(
                    out=dbg2_d[k * 128:k * 128 + nrow, :],
                    in_=db3[0:nrow, :])

    nc.finalize()
    return nc


def _decomp3(v):
    """f32 -> three bf16 arrays summing to v (error ~2^-25 |v|)."""
    v = np.asarray(v, np.float32)
    h = v.astype(NPBF)
    r = v - h.astype(np.float32)
    m = r.astype(NPBF)
    lo = (r - m.astype(np.float32)).astype(NPBF)
    return h, m, lo


def _host_prep(prob_class, predict_bbox, labels):
    pc = np.ascontiguousarray(np.asarray(prob_class, np.float32))
    pb = np.asarray(predict_bbox, np.float32)
    lab = np.asarray(labels, np.float32)

    lb = lab[..., :4] / IMG_SIZE
    cls = lab[..., 4].astype(np.int32)
    mask = (cls != NO_OBJ).astype(np.float32)

    lcx, lcy, lw, lh = (lb[..., i] for i in range(4))
    half = np.float32(0.5)
    lx0 = lcx - half * lw
    lx1 = lcx + half * lw
    ly0 = lcy - half * lh
    ly1 = lcy + half * lh
    lwE = lx1 - lx0
    lhE = ly1 - ly0
    areal = lwE * lhE

    labvals = {"lx1": lx1, "ly1": ly1, "dcx": lcx,
               "lx0": lx0, "ly0": ly0, "dcy": lcy,
               "dw": lw, "dh": lh, "mask2": 2.0 * mask,
               "uepre": areal, "ewpre": lwE, "ehpre": lhE}
    qlab = np.zeros((B, 2048), np.float32)
    for name, q in SLOT.items():
        off = (q // 3) * 512 + (q % 3) * 150
        qlab[:, off:off + T] = labvals[name]

    pcx, pcy, pw, ph = (pb[..., i] for i in range(4))
    px0 = pcx - half * pw
    px1 = pcx + half * pw
    py0 = pcy - half * ph
    py1 = pcy + half * ph
    pwE = px1 - px0
    phE = py1 - py0
    apE = pwE * phE + EPS
    presvals = {"dcx": -pcx, "dcy": -pcy, "dw": -pw, "dh": -ph,
                "uepre": apE, "ewpre": pwE, "ehpre": phE}   # [B, P] each

    scal = np.stack([px1, px0, py1, py0], axis=-1)  # [B, P, 4]

    rhsnm = np.zeros((B, 128, T), np.float32)
    bi = np.repeat(np.arange(B), T)
    ti = np.tile(np.arange(T), B)
    rhsnm[bi, cls.ravel(), ti] = np.float32(-1.0)
    rhsnm[:, 81, :] = np.float32(4.0) * mask

    pt = np.ascontiguousarray(np.transpose(pc, (0, 2, 1)))  # [B, C, P]

    qlab3 = _decomp3(qlab)                       # 3 x [B, 2048] bf16
    pres3 = {n: _decomp3(val) for n, val in presvals.items()}

    bcr = np.zeros((NCORES, NTILES, NK, 2176), NPBF)
    bcl = bcr[:, :, :, 2048:]
    for k, segs in enumerate(TILES):
        for core in range(NCORES):
            for si, (b, p0, ln, off) in enumerate(segs):
                gb = core * BL + b
                for p3 in range(3):
                    bcl[core, k, 3 * si + p3, off:off + ln] = NPBF(1.0)
                    bcr[core, k, 3 * si + p3, 0:2048] = qlab3[p3][gb]
                for i, name in enumerate(PRESUMS):
                    q = SLOT[name]
                    coff = (q // 3) * 512 + (q % 3) * 150
                    for p3 in range(3):
                        bcl[core, k, 6 + 3 * i + p3, off:off + ln] = \
                            pres3[name][p3][gb, p0:p0 + ln]
                        bcr[core, k, 6 + 3 * i + p3, coff:coff + T] = NPBF(1.0)

    in_maps = []
    for core in range(NCORES):
        bsl = slice(core * BL, (core + 1) * BL)
        in_maps.append({
            "pt": pt[bsl],
            "rhsnm": rhsnm[bsl],
            "bcr": bcr[core],
            "scal": scal[bsl].reshape(ROWS, 4),
        })
    return in_maps


def _hungarian_np(cost):
    """Jonker-Volgenant LSA fallback (same algorithm as scipy)."""
    cost = np.asarray(cost, dtype=np.float64)
    n, m = cost.shape
    INF = float("inf")
    u = np.zeros(n + 1)
    vv = np.zeros(m + 1)
    p = np.zeros(m + 1, dtype=np.int64)
    way = np.zeros(m + 1, dtype=np.int64)
    for i in range(1, n + 1):
        p[0] = i
        j0 = 0
        minv = np.full(m + 1, INF)
        used = np.zeros(m + 1, dtype=bool)
        while True:
            used[j0] = True
            i0 = p[j0]
            free = ~used[1:]
            cur = cost[i0 - 1] - u[i0] - vv[1:]
            better = free & (cur < minv[1:])
            minv[1:] = np.where(better, cur, minv[1:])
            way[1:] = np.where(better, j0, way[1:])
            masked = np.where(free, minv[1:], INF)
            j1 = int(np.argmin(masked)) + 1
            delta = masked[j1 - 1]
            uj = np.nonzero(used)[0]
            u[p[uj]] += delta
            vv[uj] -= delta
            minv[1:] = np.where(free, minv[1:] - delta, minv[1:])
            j0 = j1
            if p[j0] == 0:
                break
        while j0 != 0:
            j1 = way[j0]
            p[j0] = p[j1]
            j0 = j1
    cols = np.nonzero(p[1:])[0]
    rows = p[1:][cols] - 1
    order = np.argsort(rows)
    return rows[order], cols[order]


def _lsa(cost):
    try:
        from scipy.optimize import linear_sum_assignment
        return linear_sum_assignment(cost)
    except Exception:
        return _hungarian_np(cost)


def _install_profile_shim():
    import types

    if "antenv.axon_hooks" not in sys.modules:
        import antenv

        mod = types.ModuleType("antenv.axon_hooks")
        mod._HOOK = None
        mod.set_axon_ntff_profile_hook = lambda h: setattr(mod, "_HOOK", h)
        mod.get_axon_ntff_profile_hook = lambda: mod._HOOK
        sys.modules["antenv.axon_hooks"] = mod
        antenv.axon_hooks = mod
    import antenv.axon_hooks as ah

    if ah.get_axon_ntff_profile_hook() is None:
        try:
            from trn_agent_boot.trn_boot import _ntff_profile_via_ctypes

            hook = _ntff_profile_via_ctypes("/opt/axon/libaxon_pjrt.so")
            if hook is not None:
                ah.set_axon_ntff_profile_hook(hook)
        except Exception:
            pass
    import concourse.bass_utils as bu

    bu.upload_artifacts = lambda tmpdir: f"local:{tmpdir}"


def kernel(prob_class, predict_bbox, labels):
    global LAST_EXEC_NS, LAST_RESULTS
    if "nc" not in _CACHE:
        _CACHE["nc"] = _build_program()
    nc = _CACHE["nc"]

    in_maps = _host_prep(prob_class, predict_bbox, labels)
    if TRACE:
        _install_profile_shim()
    res = run_bass_kernel_spmd(nc, in_maps, list(range(NCORES)), trace=TRACE)
    LAST_EXEC_NS = res.exec_time_ns
    LAST_RESULTS = res

    cost = np.concatenate(
        [(res.results[i]["cost"] + res.results[i]["nllm"]).reshape(BL, P, T)
         for i in range(NCORES)], axis=0
    )

    loss = np.float32(0.0)
    cost64 = cost.astype(np.float64)
    for i in range(B):
        r, c = _lsa(cost64[i])
        loss = loss + cost[:, r, c].mean(dtype=np.float32)
    return np.float32(loss)


# revision 16
# speedup vs baseline: 1.3221x; 1.1132x over previous
"""DETR-style matching loss on 8 Trainium2 NeuronCores.

Device (data-parallel over batch, 8 samples/core): the [B,P,T] pairwise
cost matrix  cost = NLL + mask*(5*L1 + 2*GIoU_loss)  computed per core as
a [1200, 150] slab (rows = (b,p) packed, 10 tiles of 128 partitions).
Host: scipy Hungarian per sample on the device cost + the final
gather/mean (inherently sequential scalar work on 150x150 matrices).

v3 design:
- All per-(row,t) bilinear terms come from ONE bf16 3-pass PE matmul per
  PSUM bank (exact to ~2^-25): label-quantity broadcasts AND pairwise
  pre-sums (areal+apE, lwE+pwE, lhE+phE) AND pairwise pre-diffs for L1
  (lcx-pcx, ...). K=27: 6 selector rows (2 segments x hi/mid/lo) + 7
  pred-side values x hi/mid/lo against slot-indicator rows.
- NLL + 4*mask from an fp32 one-hot matmul (exact: one product/output).
- The nonlinear chain runs as 8 fused custom-DVE ops + a few stock
  tensor ops balanced across DVE/GpSimd/ScalarE.

Math (per pair, r=(b,p) row, t):
  iw = min(px1,lx1) - max(px0,lx0);  ih likewise;  inter = relu.iw*relu.ih
  ue = (areal + area_p + EPS) - inter          [= union + EPS]
  ew = (lwE + pwE) - iw;  eh = (lhE + phE) - ih   [min+max=sum identity]
  ee = ew*eh + EPS                              [= enclose + EPS]
  rc = 1/(ue*ee)   [approx recip + 1 Newton step]
  cost = [nll + 4*mask] + 2*mask*( 0.625*sum|d| - (inter*ee + ue^2)*rc )
"""

import sys

import numpy as np

for _p in ("/opt/trn_rl_repo",):
    if _p not in sys.path:
        sys.path.append(_p)

import ml_dtypes

import concourse.bacc as bacc
import concourse.tile as tile
from concourse import mybir
from concourse.bass_utils import run_bass_kernel_spmd

F32 = mybir.dt.float32
BF16 = mybir.dt.bfloat16
NPBF = ml_dtypes.bfloat16
EPS = np.float32(1e-7)
IMG_SIZE = np.float32(320.0)
NO_OBJ = 80

B, P, T, C = 64, 150, 150, 81
NCORES = 8
BL = B // NCORES          # 8 samples per core
ROWS = BL * P             # 1200 (b,p) rows per core
NTILES = (ROWS + 127) // 128   # 10
NK = 27                   # bc-matmul contraction rows

# bc slot layout: slot q at psum cols (q//3)*512 + (q%3)*150.
# chunk0 (slots 0-2) stays PSUM-resident; chunks 1-3 are copied to SBUF.
SLOT = {"lx1": 0, "ly1": 1, "dcx": 2,
        "lx0": 3, "ly0": 4, "dcy": 5,
        "dw": 6, "dh": 7, "mask2": 8,
        "uepre": 9, "ewpre": 10, "ehpre": 11}
# pred-side presum value index -> target slot (7 presums, rows 6+3i..8+3i)
PRESUMS = ["dcx", "dcy", "dw", "dh", "uepre", "ewpre", "ehpre"]


def _tile_segments():
    tiles = []
    for k in range(NTILES):
        r0, r1 = k * 128, min(k * 128 + 128, ROWS)
        segs, r = [], r0
        while r < r1:
            b = r // P
            p0 = r % P
            ln = min(r1 - r, P - p0)
            segs.append((b, p0, ln, r - r0))
            r += ln
        tiles.append(segs)
    return tiles


TILES = _tile_segments()

_CACHE = {}
TRACE = False
DEBUG = False
LAST_EXEC_NS = None
LAST_RESULTS = None


def _register_custom_ops():
    if "ops" in _CACHE:
        return _CACHE["ops"]
    from concourse import dve_ops
    from concourse.dve_spec import (
        Spec, Src0, Src1, C0, C1, C2, Zero, relu, sq, maxx, minn, lower,
        _has_src1,
    )
    from concourse.dve_uop import DveOpSpec
    from concourse.dve_table_gen import dve_ver_for

    def ref_iw(in0, in1, s0, s1, imm2):
        return np.minimum(in0, s0) - np.maximum(in1, s1)

    def ref_relumul(in0, in1, s0, s1, imm2):
        return np.maximum(in0, 0) * np.maximum(in1, 0)

    def ref_fmaeps(in0, in1, s0, s1, imm2):
        return in0 * in1 + imm2

    def ref_addsq(in0, in1, s0, s1, imm2):
        return in0 + in1 * in1

    def ref_abs2(in0, in1, s0, s1, imm2):
        return (np.abs(in0) + np.abs(in1)) * imm2

    specs = {
        "DETR_IW": Spec(body=minn(Src0, C0) - maxx(Src1, C1),
                        reference=ref_iw),
        "DETR_RELUMUL": Spec(body=relu(Src0) * relu(Src1),
                             reference=ref_relumul),
        "DETR_FMAEPS": Spec(body=Src0 * Src1 + C2, reference=ref_fmaeps),
        "DETR_ADDSQ": Spec(body=Src0 + sq(Src1), reference=ref_addsq),
        "DETR_ABS2": Spec(
            body=(maxx(Src0, Zero - Src0) + maxx(Src1, Zero - Src1)) * C2,
            reference=ref_abs2),
    }
    ops = {}
    existing = {op.name for op in dve_ops.OPS}
    for name, spec in specs.items():
        if name in existing:
            ops[name] = next(o for o in dve_ops.OPS if o.name == name)
            continue
        shas = {}
        for ver in ("v3", "v4"):
            try:
                uops = lower(spec, ver=ver)
                s = DveOpSpec(name=name, opcode=1, uops=uops,
                              rd1_en=_has_src1(spec))
                shas[ver] = s.sha(ver)
            except Exception:
                pass
        op = dve_ops.DveOp(name, spec, subdim=False, uops_sha=shas)
        dve_ops.OPS.append(op)
        dve_ops.CUSTOM_DVE_SPECS[name] = spec
        dve_ops._SUB_OPCODE_FOR_NAME[name] = (
            max(dve_ops._SUB_OPCODE_FOR_NAME.values()) + 1)
        ops[name] = op
    ops["FAST"] = dve_ops.RECIPROCAL_APPROX_FAST
    ops["NR"] = dve_ops.RECIPROCAL_APPROX_NR
    _CACHE["ops"] = ops
    return ops


def _build_program():
    ops = _register_custom_ops()
    nc = bacc.Bacc(None, target_bir_lowering=False)

    pt_d = nc.declare_dram_parameter("pt", [BL, C, P], F32, isOutput=False)
    rhs_d = nc.declare_dram_parameter("rhsnm", [BL, 128, T], F32,
                                      isOutput=False)
    bcr_d = nc.declare_dram_parameter("bcr", [NTILES, NK, 2176], BF16,
                                      isOutput=False)
    scal_d = nc.declare_dram_parameter("scal", [ROWS, 4], F32, isOutput=False)
    cost_d = nc.declare_dram_parameter("cost", [ROWS, T], F32, isOutput=True)
    nll_d = nc.declare_dram_parameter("nllm", [ROWS, T], F32, isOutput=True)
    if DEBUG:
        dbg_d = nc.declare_dram_parameter("dbg", [ROWS, 2048], F32,
                                          isOutput=True)
        dbg2_d = nc.declare_dram_parameter("dbg2", [ROWS, 3 * T], F32,
                                           isOutput=True)

    from contextlib import ExitStack

    with tile.TileContext(nc) as tc, ExitStack() as ctx:
        perb = ctx.enter_context(tc.tile_pool(name="perb", bufs=3))
        ptp = ctx.enter_context(tc.tile_pool(name="ptp", bufs=2))
        pert = ctx.enter_context(tc.tile_pool(name="pert", bufs=3))
        bcs = ctx.enter_context(tc.tile_pool(name="bcs", bufs=3))
        work = ctx.enter_context(tc.tile_pool(name="work", bufs=3))
        outp = ctx.enter_context(tc.tile_pool(name="outp", bufs=4))
        bcp0 = ctx.enter_context(tc.tile_pool(name="bcp0", bufs=2,
                                              space="PSUM"))
        bcp = ctx.enter_context(tc.tile_pool(name="bcp", bufs=1, space="PSUM"))
        nllp = ctx.enter_context(tc.tile_pool(name="nllp", bufs=1,
                                              space="PSUM"))

        lhs = {}
        rhs = {}
        def dma(out, in_):
            nc.sync.dma_start(out=out, in_=in_)

        def ensure_b(b):
            if b in lhs:
                return
            ptile = ptp.tile([C, P], F32, tag="ptile")
            dma(ptile[:], pt_d[b])
            lt = perb.tile([128, P], F32, tag="lhs")
            nc.vector.memset(lt[64:128, :], 1.0)
            nc.scalar.activation(out=lt[0:C, :], in_=ptile[:],
                                 func=mybir.ActivationFunctionType.Ln)
            rt = perb.tile([128, T], F32, tag="rhs")
            dma(rt[:], rhs_d[b])
            lhs[b] = lt
            rhs[b] = rt

        v = nc.vector
        g = nc.gpsimd
        s = nc.scalar

        for k, segs in enumerate(TILES):
            nrow = sum(sg[2] for sg in segs)
            for b, _, _, _ in segs:
                ensure_b(b)

            # ---- bc matmuls: one per PSUM bank, K=27 bf16 ----
            rt27 = pert.tile([NK, 2176], BF16, tag="rt27")
            dma(rt27[:], bcr_d[k])
            lt27 = rt27[:, 2048:2176]
            ps0 = bcp0.tile([128, 512], F32, tag="bcps0")
            nc.tensor.matmul(ps0[:, :], lt27, rt27[:, 0:512],
                             start=True, stop=True)
            ps = bcp.tile([128, 1536], F32, tag="bcps")
            for j in range(3):
                nc.tensor.matmul(ps[:, j * 512:(j + 1) * 512], lt27,
                                 rt27[:, (j + 1) * 512:(j + 2) * 512],
                                 start=True, stop=True)

            # copy chunks 1-3 to SBUF (split DVE / ACT) — frees ps for k+1
            sb = bcs.tile([128, 2048], F32, tag="bcsb")
            v.tensor_copy(sb[:, 512:1024], ps[:, 0:512])
            s.copy(sb[:, 1024:2048], ps[:, 512:1536])

            def bcP(name):
                q = SLOT[name]
                off = (q % 3) * 150
                return ps0[0:nrow, off:off + T]

            def bcS(name):
                q = SLOT[name]
                off = (q // 3) * 512 + (q % 3) * 150
                return sb[0:nrow, off:off + T]

            # ---- nll + 4*mask (fp32 one-hot matmul per segment) ----
            for si, (b, p0, ln, off) in enumerate(segs):
                nm = nllp.tile([128, 512], F32, tag=f"nll{si}",
                               name=f"nll{si}")
                nc.tensor.matmul(nm[0:ln, 0:T], lhs[b][:, p0:p0 + ln],
                                 rhs[b][:, :], start=True, stop=True)
                nsb = outp.tile([128, T], F32, tag=f"nsb{si}",
                                name=f"nsb{si}")
                s.copy(out=nsb[0:ln, :], in_=nm[0:ln, 0:T])
                dma(nll_d[k * 128 + off:k * 128 + off + ln, :],
                    nsb[0:ln, :])

            # ---- pred xyxy scalars ----
            sc = pert.tile([128, 4], F32, tag="scal")
            dma(sc[0:nrow, :], scal_d[k * 128:k * 128 + nrow, :])
            px1, px0, py1, py0 = (sc[0:nrow, i:i + 1] for i in range(4))

            def wt(tag, w=T):
                return work.tile([128, w], F32, tag=tag, name=tag)

            # ---- fused chain ----
            # t3 = [inter | iw | ih] adjacent for the GPS triple-subtract
            t3 = wt("t3", 3 * T)
            iw = t3[0:nrow, T:2 * T]
            ih = t3[0:nrow, 2 * T:3 * T]
            inter = t3[0:nrow, 0:T]
            v._custom_dve(ops["DETR_IW"], out=iw, in0=bcP("lx1"),
                          in1=bcS("lx0"), s0=px1, s1=px0)
            v._custom_dve(ops["DETR_IW"], out=ih, in0=bcP("ly1"),
                          in1=bcS("ly0"), s0=py1, s1=py0)
            v._custom_dve(ops["DETR_RELUMUL"], out=inter, in0=iw, in1=ih)

            # [ue|ew|eh] = [uepre|ewpre|ehpre] - [inter|iw|ih]  (one GPS op)
            uew = wt("uew", 3 * T)
            g.tensor_sub(uew[0:nrow, :], sb[0:nrow, 1536:1536 + 3 * T],
                         t3[0:nrow, :])
            ue = uew[0:nrow, 0:T]
            ew = uew[0:nrow, T:2 * T]
            eh = uew[0:nrow, 2 * T:3 * T]

            ee = wt("ee")[0:nrow, :]
            v._custom_dve(ops["DETR_FMAEPS"], out=ee, in0=ew, in1=eh,
                          imm2=float(EPS))
            prod = wt("prod")[0:nrow, :]
            v.tensor_mul(prod, ue, ee)
            t1 = wt("t1")[0:nrow, :]
            v.tensor_mul(t1, inter, ee)
            r0 = wt("r0")[0:nrow, :]
            rc = wt("rc")[0:nrow, :]
            v.reciprocal_approx_accurate(out=rc, in_=prod, scratch=r0)
            gnum = wt("gnum")[0:nrow, :]
            v._custom_dve(ops["DETR_ADDSQ"], out=gnum, in0=t1, in1=ue)
            gterm = wt("gterm")[0:nrow, :]
            g.tensor_mul(gterm, gnum, rc)

            s01 = wt("s01")[0:nrow, :]
            v._custom_dve(ops["DETR_ABS2"], out=s01, in0=bcP("dcx"),
                          in1=bcS("dcy"), imm2=0.625)
            s23 = wt("s23")[0:nrow, :]
            v._custom_dve(ops["DETR_ABS2"], out=s23, in0=bcS("dw"),
                          in1=bcS("dh"), imm2=0.625)
            l1t = wt("l1t")[0:nrow, :]
            g.tensor_add(l1t, s01, s23)
            pre = wt("pre")[0:nrow, :]
            g.tensor_sub(pre, l1t, gterm)
            mp = outp.tile([128, T], F32, tag="mp")
            g.tensor_mul(mp[0:nrow, :], pre, bcS("mask2"))

            dma(cost_d[k * 128:k * 128 + nrow, :], mp[0:nrow, :])
            if DEBUG:
                dbt = outp.tile([128, 512], F32, tag="dbt")
                s.copy(out=dbt[0:nrow, :], in_=ps[0:nrow, 0:512])
                nc.sync.dma_start(
                    out=dbg_d[k * 128:k * 128 + nrow, 0:512],
                    in_=dbt[0:nrow, :])
                nc.sync.dma_start(
                    out=dbg_d[k * 128:k * 128 + nrow, 512:2048],
                    in_=sb[0:nrow, 512:2048])
                db3 = outp.tile([128, 3 * T], F32, tag="db3")
                s.copy(out=db3[0:nrow, 0:T], in_=ee)
                s.copy(out=db3[0:nrow, T:2 * T], in_=rc)
                s.copy(out=db3[0:nrow, 2 * T:3 * T], in_=gterm)
                nc.sync.dma_start(
                    out=dbg2_d[k * 128:k * 128 + nrow, :],
                    in_=db3[0:nrow, :])

    nc.finalize()
    return nc


def _decomp3(v):
    """f32 -> three bf16 arrays summing to v (error ~2^-25 |v|)."""
    v = np.asarray(v, np.float32)
    h = v.astype(NPBF)
    r = v - h.astype(np.float32)
    m = r.astype(NPBF)
    lo = (r - m.astype(np.float32)).astype(NPBF)
    return h, m, lo


def _host_prep(prob_class, predict_bbox, labels):
    pc = np.ascontiguousarray(np.asarray(prob_class, np.float32))
    pb = np.asarray(predict_bbox, np.float32)
    lab = np.asarray(labels, np.float32)

    lb = lab[..., :4] / IMG_SIZE
    cls = lab[..., 4].astype(np.int32)
    mask = (cls != NO_OBJ).astype(np.float32)

    lcx, lcy, lw, lh = (lb[..., i] for i in range(4))
    half = np.float32(0.5)
    lx0 = lcx - half * lw
    lx1 = lcx + half * lw
    ly0 = lcy - half * lh
    ly1 = lcy + half * lh
    lwE = lx1 - lx0
    lhE = ly1 - ly0
    areal = lwE * lhE

    labvals = {"lx1": lx1, "ly1": ly1, "dcx": lcx,
               "lx0": lx0, "ly0": ly0, "dcy": lcy,
               "dw": lw, "dh": lh, "mask2": 2.0 * mask,
               "uepre": areal, "ewpre": lwE, "ehpre": lhE}
    qlab = np.zeros((B, 2048), np.float32)
    for name, q in SLOT.items():
        off = (q // 3) * 512 + (q % 3) * 150
        qlab[:, off:off + T] = labvals[name]

    pcx, pcy, pw, ph = (pb[..., i] for i in range(4))
    px0 = pcx - half * pw
    px1 = pcx + half * pw
    py0 = pcy - half * ph
    py1 = pcy + half * ph
    pwE = px1 - px0
    phE = py1 - py0
    apE = pwE * phE + EPS
    presvals = {"dcx": -pcx, "dcy": -pcy, "dw": -pw, "dh": -ph,
                "uepre": apE, "ewpre": pwE, "ehpre": phE}   # [B, P] each

    scal = np.stack([px1, px0, py1, py0], axis=-1)  # [B, P, 4]

    rhsnm = np.zeros((B, 128, T), np.float32)
    bi = np.repeat(np.arange(B), T)
    ti = np.tile(np.arange(T), B)
    rhsnm[bi, cls.ravel(), ti] = np.float32(-1.0)
    rhsnm[:, 81, :] = np.float32(4.0) * mask

    pt = np.ascontiguousarray(np.transpose(pc, (0, 2, 1)))  # [B, C, P]

    qlab3 = _decomp3(qlab)                       # 3 x [B, 2048] bf16
    pres3 = {n: _decomp3(val) for n, val in presvals.items()}

    bcr = np.zeros((NCORES, NTILES, NK, 2176), NPBF)
    bcl = bcr[:, :, :, 2048:]
    for k, segs in enumerate(TILES):
        for core in range(NCORES):
            for si, (b, p0, ln, off) in enumerate(segs):
                gb = core * BL + b
                for p3 in range(3):
                    bcl[core, k, 3 * si + p3, off:off + ln] = NPBF(1.0)
                    bcr[core, k, 3 * si + p3, 0:2048] = qlab3[p3][gb]
                for i, name in enumerate(PRESUMS):
                    q = SLOT[name]
                    coff = (q // 3) * 512 + (q % 3) * 150
                    for p3 in range(3):
                        bcl[core, k, 6 + 3 * i + p3, off:off + ln] = \
                            pres3[name][p3][gb, p0:p0 + ln]
                        bcr[core, k, 6 + 3 * i + p3, coff:coff + T] = NPBF(1.0)

    in_maps = []
    for core in range(NCORES):
        bsl = slice(core * BL, (core + 1) * BL)
        in_maps.append({
            "pt": pt[bsl],
            "rhsnm": rhsnm[bsl],
            "bcr": bcr[core],
            "scal": scal[bsl].reshape(ROWS, 4),
        })
    return in_maps


def _hungarian_np(cost):
    """Jonker-Volgenant LSA fallback (same algorithm as scipy)."""
    cost = np.asarray(cost, dtype=np.float64)
    n, m = cost.shape
    INF = float("inf")
    u = np.zeros(n + 1)
    vv = np.zeros(m + 1)
    p = np.zeros(m + 1, dtype=np.int64)
    way = np.zeros(m + 1, dtype=np.int64)
    for i in range(1, n + 1):
        p[0] = i
        j0 = 0
        minv = np.full(m + 1, INF)
        used = np.zeros(m + 1, dtype=bool)
        while True:
            used[j0] = True
            i0 = p[j0]
            free = ~used[1:]
            cur = cost[i0 - 1] - u[i0] - vv[1:]
            better = free & (cur < minv[1:])
            minv[1:] = np.where(better, cur, minv[1:])
            way[1:] = np.where(better, j0, way[1:])
            masked = np.where(free, minv[1:], INF)
            j1 = int(np.argmin(masked)) + 1
            delta = masked[j1 - 1]
            uj = np.nonzero(used)[0]
            u[p[uj]] += delta
            vv[uj] -= delta
            minv[1:] = np.where(free, minv[1:] - delta, minv[1:])
            j0 = j1
            if p[j0] == 0:
                break
        while j0 != 0:
            j1 = way[j0]
            p[j0] = p[j1]
            j0 = j1
    cols = np.nonzero(p[1:])[0]
    rows = p[1:][cols] - 1
    order = np.argsort(rows)
    return rows[order], cols[order]


def _lsa(cost):
    try:
        from scipy.optimize import linear_sum_assignment
        return linear_sum_assignment(cost)
    except Exception:
        return _hungarian_np(cost)


def _install_profile_shim():
    import types

    if "antenv.axon_hooks" not in sys.modules:
        import antenv

        mod = types.ModuleType("antenv.axon_hooks")
        mod._HOOK = None
        mod.set_axon_ntff_profile_hook = lambda h: setattr(mod, "_HOOK", h)
        mod.get_axon_ntff_profile_hook = lambda: mod._HOOK
        sys.modules["antenv.axon_hooks"] = mod
        antenv.axon_hooks = mod
    import antenv.axon_hooks as ah

    if ah.get_axon_ntff_profile_hook() is None:
        try:
            from trn_agent_boot.trn_boot import _ntff_profile_via_ctypes

            hook = _ntff_profile_via_ctypes("/opt/axon/libaxon_pjrt.so")
            if hook is not None:
                ah.set_axon_ntff_profile_hook(hook)
        except Exception:
            pass
    import concourse.bass_utils as bu

    bu.upload_artifacts = lambda tmpdir: f"local:{tmpdir}"


def kernel(prob_class, predict_bbox, labels):
    global LAST_EXEC_NS, LAST_RESULTS
    if "nc" not in _CACHE:
        _CACHE["nc"] = _build_program()
    nc = _CACHE["nc"]

    in_maps = _host_prep(prob_class, predict_bbox, labels)
    if TRACE:
        _install_profile_shim()
    res = run_bass_kernel_spmd(nc, in_maps, list(range(NCORES)), trace=TRACE)
    LAST_EXEC_NS = res.exec_time_ns
    LAST_RESULTS = res

    cost = np.concatenate(
        [(res.results[i]["cost"] + res.results[i]["nllm"]).reshape(BL, P, T)
         for i in range(NCORES)], axis=0
    )

    loss = np.float32(0.0)
    cost64 = cost.astype(np.float64)
    for i in range(B):
        r, c = _lsa(cost64[i])
        loss = loss + cost[:, r, c].mean(dtype=np.float32)
    return np.float32(loss)
